# revision 5
# baseline (speedup 1.0000x reference)
"""AttentionGCNLayer Trainium2 kernel (v2).

Per-sample computation (B=8 samples -> 8 NeuronCores, data-parallel):
  identity = x @ W_it + b_it
  gcn      = relu(adj @ (x @ W_g + b_g))
  h        = LN1(identity + gcn)
  attn     = MHSA(h)  (8 heads, D=32)
  out      = LN2(h + attn)

Key design points (vs the earlier 413us baseline):
  - Host-side weight folding: LN1's gamma folds into W_q/W_k/W_v rows; the
    k-bias drops entirely (softmax is invariant to per-query shifts); the
    v-bias folds into the output-projection bias (softmax rows sum to 1).
    All weights are pre-cast to bf16 on the host, so every device matmul and
    transpose is single-pass bf16 (no fp32 LOW_HIGH).
  - Biases enter via K=1 "broadcast" matmuls that open each PSUM accumulation
    group, so PSUM->SBUF drains are plain copies (ScalarE Copy, no DVE
    tensor_tensor with broadcast operands, no gpsimd broadcast DMAs).
  - Softmax exp is split across ScalarE (table exp) and VectorE (Schraudolph
    bit-trick: i16 = round(score * 128*scale/ln2 + (127*128 - sigma)),
    bitcast i16 -> bf16 ~= exp(score*scale)). Softmax normalization cancels
    the approximation's mean bias; measured end-to-end error ~1.7e-3.
  - Attention inner loop is software-pipelined: scores(k+1) is emitted before
    attnV(k) so the PE streams during the exp of chunk k, and exp alternates
    ScalarE/DVE per head-pair so both engines chew concurrently.
  - qkv projections run inside the per-token-chunk loop right after LN1, so
    attention for the first token half starts as early as possible.
  - scoresT layout [k-tokens on partition, q on free] as before: softmax
    denominators via ones-matmul, attn@V with V stationary + column tiling.
"""

import sys

sys.path.insert(0, "/opt/trn_rl_repo")

import numpy as np

import concourse.bass as bass
import concourse.tile as tile
from concourse import bacc, mybir
from concourse.bass_utils import run_bass_kernel_spmd
from concourse.masks import make_identity

F32 = mybir.dt.float32
BF16 = mybir.dt.bfloat16
I16 = mybir.dt.int16
I32 = mybir.dt.int32
AF = mybir.ActivationFunctionType
ALU = mybir.AluOpType

B, N, CI, CO, H, D = 8, 1024, 128, 256, 8, 32
P = 128
MT = N // P  # 8 token chunks
EPS = 1e-5
SCALE = float(1.0 / np.sqrt(np.float32(D)))
NCORES = 8
MAGIC_P1 = 0x5F3759DF + 1  # quake rsqrt magic + 1 (for the ~t + (M+1) form)

# Schraudolph constants: bf16 bit pattern of exp(scale*x) via int16 affine.
EXP_A = float(SCALE * 128.0 / np.log(2.0))
EXP_B = float(127 * 128 - 9.0)

# which exp tiles go to the DVE: (tp == 1) and k in this set (per qh,g)
DVE_EXP_KS = (0, 1, 2, 4, 5, 6)


def _rsqrt_dve(nc, pool, var_ap, out_ap, consts, n, tag):
    """out = 1/sqrt(var + eps) on VectorE only, batched over [128, n].

    Quake bit-trick seed + 2 Newton iterations (~5e-6 rel err). Keeps
    ScalarE free of Ln/Sqrt so its activation table never switches.
    """
    eps_sb, sh1_i, neg1_i, magic_i = consts
    xe = pool.tile([P, n], F32, tag=f"rs_xe{tag}")
    nc.vector.tensor_scalar_add(xe, var_ap, eps_sb)
    y = pool.tile([P, n], F32, tag=f"rs_y{tag}")
    ti = pool.tile([P, n], I32, tag=f"rs_ti{tag}")
    # ~(x >> 1)
    nc.vector.tensor_scalar(
        out=ti, in0=xe.bitcast(I32), scalar1=sh1_i, scalar2=neg1_i,
        op0=ALU.logical_shift_right, op1=ALU.bitwise_xor)
    # + (MAGIC+1)  ==  MAGIC - (x >> 1)
    nc.vector.tensor_tensor(
        out=y.bitcast(I32), in0=ti, in1=magic_i.to_broadcast((P, n)), op=ALU.add)
    h = pool.tile([P, n], F32, tag=f"rs_h{tag}")
    nc.vector.tensor_scalar_mul(h, xe, 0.5)
    t2 = pool.tile([P, n], F32, tag=f"rs_t2{tag}")
    for _ in range(2):
        nc.vector.tensor_mul(t2, y, y)
        nc.vector.tensor_mul(t2, t2, h)
        nc.vector.tensor_scalar(
            out=t2, in0=t2, scalar1=-1.0, scalar2=1.5, op0=ALU.mult, op1=ALU.add)
        nc.vector.tensor_mul(y, y, t2)
    nc.vector.tensor_copy(out_ap, y)


def build_bass(trivial1=True, trivial2=True):
    nc = bacc.Bacc()

    x_d = nc.dram_tensor("x", (N, CI), F32, kind="ExternalInput")
    adj_d = nc.dram_tensor("adj", (N, N), F32, kind="ExternalInput")
    wit_d = nc.dram_tensor("wit", (CI, CO), BF16, kind="ExternalInput")
    wg_d = nc.dram_tensor("wg", (CI, CO), BF16, kind="ExternalInput")
    wq_d = nc.dram_tensor("wq", (CO, CO), BF16, kind="ExternalInput")
    wk_d = nc.dram_tensor("wk", (CO, CO), BF16, kind="ExternalInput")
    wv_d = nc.dram_tensor("wv", (CO, CO), BF16, kind="ExternalInput")
    wo_d = nc.dram_tensor("wo", (CO, CO), BF16, kind="ExternalInput")
    bit_d = nc.dram_tensor("bit", (CO,), BF16, kind="ExternalInput")
    bg_d = nc.dram_tensor("bg", (CO,), BF16, kind="ExternalInput")
    bq_d = nc.dram_tensor("bq", (CO,), BF16, kind="ExternalInput")
    bb2_d = nc.dram_tensor("bb2", (CO,), BF16, kind="ExternalInput")
    if not trivial1:
        g1_d = nc.dram_tensor("g1v", (CO,), F32, kind="ExternalInput")
    if not trivial2:
        g2_d = nc.dram_tensor("g2v", (CO,), F32, kind="ExternalInput")
        be2_d = nc.dram_tensor("be2v", (CO,), F32, kind="ExternalInput")
    out_d = nc.dram_tensor("out", (N, CO), F32, kind="ExternalOutput")

    with tile.TileContext(nc) as tc:
        from contextlib import ExitStack

        with ExitStack() as ctx:
            singles = ctx.enter_context(tc.tile_pool(name="singles", bufs=1))
            stemp = ctx.enter_context(tc.tile_pool(name="stemp", bufs=3))
            ptemp = ctx.enter_context(tc.tile_pool(name="ptemp", bufs=5))
            adj_pool = ctx.enter_context(tc.tile_pool(name="adj", bufs=3))
            adjb_pool = ctx.enter_context(tc.tile_pool(name="adjb", bufs=2))
            adjT_pool = ctx.enter_context(tc.tile_pool(name="adjT", bufs=2))
            expT_pool = ctx.enter_context(tc.tile_pool(name="expT", bufs=6))
            ytile_pool = ctx.enter_context(tc.tile_pool(name="ytile", bufs=2))

            # ---------------- Phase -1: start the big input DMAs first ------
            adj_r = adj_d[:].rearrange("(mt p) k -> p mt k", p=P)
            x_sb = singles.tile([P, MT, CI], F32)
            adj_tiles = []
            for m in range(3):
                ab = adj_pool.tile([P, N], F32, tag="ab")
                nc.sync.dma_start(ab, adj_r[:, m, :])
                adj_tiles.append(ab)
            nc.sync.dma_start(x_sb, x_d[:].rearrange("(mt p) c -> p mt c", p=P))

            # ---------------- Phase 0: constants / weights ----------------
            identB = singles.tile([P, P], BF16)
            make_identity(nc, identB)
            ones_sb = singles.tile([P, D], BF16)
            nc.vector.memset(ones_sb, 1.0)
            ones1 = singles.tile([1, 512], BF16)
            nc.vector.memset(ones1, 1.0)
            eps_sb = singles.tile([P, 1], F32)
            nc.vector.memset(eps_sb, EPS)
            sh1_i = singles.tile([P, 1], I32)
            nc.vector.memset(sh1_i, 1)
            neg1_i = singles.tile([P, 1], I32)
            nc.vector.memset(neg1_i, -1)
            magic_i = singles.tile([P, 1], I32)
            nc.vector.memset(magic_i, MAGIC_P1)
            consts = (eps_sb, sh1_i, neg1_i, magic_i)

            wit_sb = singles.tile([P, CO], BF16)
            nc.sync.dma_start(wit_sb, wit_d[:])
            wg_sb = singles.tile([P, CO], BF16)
            nc.sync.dma_start(wg_sb, wg_d[:])

            def load_w2(dram, name):  # [256,256] -> [128, 2, 256]
                t = singles.tile([P, 2, CO], BF16, tag=f"w2_{name}")
                nc.sync.dma_start(
                    t, dram[:].rearrange("(ko ki) n -> ki ko n", ki=P))
                return t

            wq_sb = load_w2(wq_d, "wq")
            wk_sb = load_w2(wk_d, "wk")
            wv_sb = load_w2(wv_d, "wv")
            wo_sb = load_w2(wo_d, "wo")

            def load_row(dram, name):  # [256] -> [1, 256] single-partition row
                t = singles.tile([1, CO], BF16, tag=f"row_{name}")
                nc.sync.dma_start(t, dram[:].rearrange("(a c) -> a c", a=1))
                return t

            bit_row = load_row(bit_d, "bit")
            bg_row = load_row(bg_d, "bg")
            bq_row = load_row(bq_d, "bq")
            bb2_row = load_row(bb2_d, "bb2")

            def load_bc(dram, name):  # broadcast along partitions: [128, 256]
                t = singles.tile([P, CO], F32, tag=f"bc_{name}")
                src = dram[:]
                bcast = bass.AP(tensor=src.tensor, offset=src.offset,
                                ap=[[0, P]] + list(src.ap))
                nc.gpsimd.dma_start(out=t, in_=bcast)
                return t

            if not trivial1:
                g1_bc = load_bc(g1_d, "g1")
            if not trivial2:
                g2_bc = load_bc(g2_d, "g2")
                be2_bc = load_bc(be2_d, "be2")

            # persistent activations
            x_bf = singles.tile([P, MT, CI], BF16)
            xT_bf = singles.tile([P, MT, P], BF16)       # x^T  [ci, m] bf16
            t_sb = singles.tile([P, MT, CO], BF16)       # x@W_g + b_g [tok, c]
            s_all = singles.tile([P, MT, CO], F32)       # pre-LN1 residual
            h_sb = singles.tile([P, MT, CO], F32)        # normalized hhat
            h_bf = singles.tile([P, MT, CO], BF16)       # hhat bf16 (for h^T)
            mv_all = singles.tile([P, MT, 2], F32)       # LN1 mean/var
            rstd_all = singles.tile([P, MT], F32)        # LN1 rstd
            hT_sb = singles.tile([P, 2, N], BF16)        # hhat^T  [c, tok]
            qT_sb = singles.tile([P, 2, N], BF16)        # q^T     [c, tok]
            kT_sb = singles.tile([P, 2, N], BF16)        # k^T     [c, tok]
            v_sb = singles.tile([P, MT, CO], BF16)       # v       [tok, c]
            outT_sb = singles.tile([P, 2, N], BF16)      # attn-out^T [c, tok]

            with ExitStack() as pre:
                tr_ps = pre.enter_context(
                    tc.tile_pool(name="tr_ps", bufs=2, space="PSUM"))
                htr_ps = pre.enter_context(
                    tc.tile_pool(name="htr_ps", bufs=1, space="PSUM"))
                mm_ps = pre.enter_context(
                    tc.tile_pool(name="mm_ps", bufs=2, space="PSUM"))
                qk_ps = pre.enter_context(
                    tc.tile_pool(name="qk_ps", bufs=2, space="PSUM"))

                # Warm-up transpose so PE observes the gpsimd sem early.
                warm_ps = tr_ps.tile([P, 4, P], BF16, tag="tr")
                nc.tensor.transpose(warm_ps[:, 0, :], identB, identB)

                # ---------------- Phase 1: x -> bf16 -> transpose ------------
                nc.vector.tensor_copy(x_bf, x_sb)
                for half in range(2):
                    ps = tr_ps.tile([P, 4, P], BF16, tag="tr")
                    for i in range(4):
                        m = half * 4 + i
                        nc.tensor.transpose(ps[:, i, :], x_bf[:, m, :], identB)
                    nc.scalar.copy(xT_bf[:, half * 4:half * 4 + 4, :], ps)

                # ---------------- Phase 2: t = x@W_g + b_g (bf16) ------------
                for m in range(MT):
                    tp = mm_ps.tile([P, CO], F32, tag="mm256")
                    nc.tensor.matmul(tp, ones1[:, 0:P], bg_row,
                                     start=True, stop=False)
                    nc.tensor.matmul(tp, xT_bf[:, m, :], wg_sb,
                                     start=False, stop=True)
                    nc.scalar.copy(t_sb[:, m, :], tp)

                # ---------------- Phase 3: per-chunk gcn + LN1 + qkv ---------
                for m in range(MT):
                    if m < 3:
                        ab = adj_tiles[m]
                    else:
                        ab = adj_pool.tile([P, N], F32, tag="ab")
                        nc.sync.dma_start(ab, adj_r[:, m, :])
                    ab_bf = adjb_pool.tile([P, N], BF16, tag="ab_bf")
                    nc.vector.tensor_copy(ab_bf, ab)
                    at = adjT_pool.tile([P, MT, P], BF16)
                    for half in range(2):
                        ps = tr_ps.tile([P, 4, P], BF16, tag="tr")
                        for i in range(4):
                            k = half * 4 + i
                            nc.tensor.transpose(
                                ps[:, i, :], ab_bf[:, k * P:(k + 1) * P], identB)
                        nc.scalar.copy(at[:, half * 4:half * 4 + 4, :], ps)
                    # identity path for this chunk
                    ip = mm_ps.tile([P, CO], F32, tag="mm256")
                    nc.tensor.matmul(ip, ones1[:, 0:P], bit_row,
                                     start=True, stop=False)
                    nc.tensor.matmul(ip, xT_bf[:, m, :], wit_sb,
                                     start=False, stop=True)
                    id_sb = stemp.tile([P, CO], F32, tag="id_sb")
                    nc.scalar.copy(id_sb, ip)
                    # gcn chunk
                    gp = mm_ps.tile([P, CO], F32, tag="mm256")
                    for k in range(MT):
                        nc.tensor.matmul(gp, at[:, k, :], t_sb[:, k, :],
                                         start=(k == 0), stop=(k == MT - 1))
                    # s = identity + relu(gcn)
                    nc.vector.scalar_tensor_tensor(
                        out=s_all[:, m, :], in0=gp, scalar=0.0,
                        in1=id_sb, op0=ALU.max, op1=ALU.add)
                    stats = stemp.tile([P, 6], F32, tag="ln_stats")
                    nc.vector.bn_stats(out=stats, in_=s_all[:, m, :])
                    nc.vector.bn_aggr(out=mv_all[:, m, :], in_=stats)
                    _rsqrt_dve(nc, stemp, mv_all[:, m, 1:2],
                               rstd_all[:, m:m + 1], consts, 1, "a")
                    # normalize (hhat; LN1 affine folded into q/k/v weights)
                    nc.vector.tensor_scalar(
                        out=h_sb[:, m, :], in0=s_all[:, m, :],
                        scalar1=mv_all[:, m, 0:1], scalar2=rstd_all[:, m:m + 1],
                        op0=ALU.subtract, op1=ALU.mult)
                    nc.vector.tensor_copy(h_bf[:, m, :], h_sb[:, m, :])
                    # hhat^T (bf16) for this chunk
                    ps = htr_ps.tile([P, 2, P], BF16, tag="htr")
                    nc.tensor.transpose(ps[:, 0, :], h_bf[:, m, 0:P], identB)
                    nc.tensor.transpose(ps[:, 1, :], h_bf[:, m, P:CO], identB)
                    nc.vector.tensor_copy(hT_sb[:, :, m * P:(m + 1) * P], ps)
                    # q^T / k^T for this chunk (c-major), v (tok-major)
                    msl = slice(m * P, (m + 1) * P)
                    qkp = qk_ps.tile([P, 4, P], F32, tag="qk")
                    for oc in range(2):
                        nc.tensor.matmul(
                            qkp[:, oc, :], bq_row[:, oc * P:(oc + 1) * P],
                            ones1[:, 0:P], start=True, stop=False,
                            skip_group_check=True)
                        for kc in range(2):
                            nc.tensor.matmul(
                                qkp[:, oc, :], wq_sb[:, kc, oc * P:(oc + 1) * P],
                                hT_sb[:, kc, msl],
                                start=False, stop=(kc == 1),
                                skip_group_check=True)
                    for oc in range(2):
                        for kc in range(2):
                            nc.tensor.matmul(
                                qkp[:, 2 + oc, :],
                                wk_sb[:, kc, oc * P:(oc + 1) * P],
                                hT_sb[:, kc, msl],
                                start=(kc == 0), stop=(kc == 1),
                                skip_group_check=True)
                    nc.scalar.copy(qT_sb[:, :, msl], qkp[:, 0:2, :])
                    nc.scalar.copy(kT_sb[:, :, msl], qkp[:, 2:4, :])
                    vp = mm_ps.tile([P, CO], F32, tag="mm256")
                    for kc in range(2):
                        nc.tensor.matmul(vp, hT_sb[:, kc, msl], wv_sb[:, kc, :],
                                         start=(kc == 0), stop=(kc == 1))
                    nc.scalar.copy(v_sb[:, m, :], vp)

            # ---------------- Phase 5+6: attention + output ----------------
            with ExitStack() as att:
                sc_ps = att.enter_context(
                    tc.tile_pool(name="sc_ps", bufs=2, space="PSUM"))
                acc_ps = att.enter_context(
                    tc.tile_pool(name="acc_ps", bufs=1, space="PSUM"))
                proj_ps = att.enter_context(
                    tc.tile_pool(name="proj_ps", bufs=2, space="PSUM"))

                def proj_ln2_store(qh):
                    """Projection + residual + LN2 + DMA for 4 token chunks."""
                    s2s = []
                    mv2 = ptemp.tile([P, 4, 2], F32, tag="mv2")
                    for i in range(4):
                        m = qh * 4 + i
                        pp = proj_ps.tile([P, CO], F32, tag="proj")
                        nc.tensor.matmul(pp, ones1[:, 0:P], bb2_row,
                                         start=True, stop=False)
                        for cc in range(2):
                            nc.tensor.matmul(
                                pp, outT_sb[:, cc, m * P:(m + 1) * P],
                                wo_sb[:, cc, :],
                                start=False, stop=(cc == 1))
                        # s2 = h*g1 + proj + bb2  (bb2 already in psum)
                        s2 = ptemp.tile([P, CO], F32, tag=f"s2_{i}")
                        if trivial1:
                            nc.vector.tensor_add(s2, pp, h_sb[:, m, :])
                        else:
                            nc.vector.tensor_mul(s2, h_sb[:, m, :], g1_bc)
                            nc.vector.tensor_add(s2, s2, pp)
                        stats = ptemp.tile([P, 6], F32, tag="ln_stats2")
                        nc.vector.bn_stats(out=stats, in_=s2)
                        nc.vector.bn_aggr(out=mv2[:, i, :], in_=stats)
                        s2s.append(s2)
                    rstd2 = ptemp.tile([P, 4], F32, tag="rstd2")
                    _rsqrt_dve(nc, ptemp, mv2[:, :, 1], rstd2, consts, 4, "b")
                    for i in range(4):
                        m = qh * 4 + i
                        yt = ytile_pool.tile([P, CO], F32)
                        nc.vector.tensor_scalar(
                            out=yt, in0=s2s[i],
                            scalar1=mv2[:, i, 0:1], scalar2=rstd2[:, i:i + 1],
                            op0=ALU.subtract, op1=ALU.mult)
                        if not trivial2:
                            nc.vector.tensor_mul(yt, yt, g2_bc)
                            nc.vector.tensor_add(yt, yt, be2_bc)
                        nc.sync.dma_start(
                            out_d[:].rearrange("(mt p) c -> p mt c", p=P)[:, m, :],
                            yt)

                for qh in range(2):
                    qsl = slice(qh * 512, (qh + 1) * 512)
                    for g in range(2):
                        outb = acc_ps.tile([P, 512], F32, tag="outb")
                        denb = acc_ps.tile([P, 512], F32, tag="denb")

                        def av_den(k, exs):
                            # attn@V + denominators, interleaved so adjacent
                            # matmuls hit different PSUM tensors AND different
                            # column groups (lets the PE overlap them).
                            for tp in range(2):
                                for j2 in range(2):
                                    hh = 4 * g + 2 * tp + j2
                                    hs = 4 * g + 2 * tp + (1 - j2)  # swapped
                                    cp = 32 * (hh % 4)
                                    cps = 32 * (hs % 4)
                                    esl = slice(j2 * 512, (j2 + 1) * 512)
                                    esls = slice((1 - j2) * 512, (2 - j2) * 512)
                                    nc.tensor.matmul(
                                        outb[cp:cp + 32, :],
                                        v_sb[:, k, hh * D:(hh + 1) * D],
                                        exs[tp][:, esl],
                                        start=(k == 0), stop=(k == MT - 1),
                                        tile_position=(0, cp),
                                        skip_group_check=True)
                                    nc.tensor.matmul(
                                        denb[cps:cps + 32, :],
                                        ones_sb,
                                        exs[tp][:, esls],
                                        start=(k == 0), stop=(k == MT - 1),
                                        tile_position=(0, cps),
                                        skip_group_check=True)

                        prev_exs = None
                        for k in range(MT):
                            exs = []
                            for tp in range(2):  # head pair within group
                                sc = sc_ps.tile([P, 1024], F32, tag="sc")
                                for j2 in range(2):
                                    hh = 4 * g + 2 * tp + j2   # global head
                                    bp = 32 * (hh % 4)
                                    nc.tensor.matmul(
                                        sc[:, j2 * 512:(j2 + 1) * 512],
                                        kT_sb[bp:bp + 32, g, k * P:(k + 1) * P],
                                        qT_sb[bp:bp + 32, g, qsl],
                                        start=True, stop=True,
                                        tile_position=(bp, 0))
                                if tp == 1 and k in DVE_EXP_KS:
                                    exi = expT_pool.tile([P, 1024], I16,
                                                         tag="exi")
                                    nc.vector.tensor_scalar(
                                        out=exi, in0=sc,
                                        scalar1=EXP_A, scalar2=EXP_B,
                                        op0=ALU.mult, op1=ALU.add)
                                    exs.append(exi.bitcast(BF16))
                                else:
                                    ex = expT_pool.tile([P, 1024], BF16,
                                                        tag="ex")
                                    nc.scalar.activation(ex, sc, AF.Exp,
                                                         scale=SCALE)
                                    exs.append(ex)
                            if prev_exs is not None:
                                av_den(k - 1, prev_exs)
                            prev_exs = exs
                        av_den(MT - 1, prev_exs)

                        rec = stemp.tile([P, 512], F32, tag="rec")
                        nc.vector.reciprocal_approx_fast(out=rec, in_=denb)
                        nc.vector.tensor_mul(outT_sb[:, g, qsl], outb, rec)
                    # both head groups of this token half done -> drain output
                    proj_ln2_store(qh)

    nc.finalize()
    return nc


_CACHE = {}


def _get_nc(trivial1, trivial2):
    key = (trivial1, trivial2)
    if key not in _CACHE:
        _CACHE[key] = build_bass(*key)
    return _CACHE[key]


def _prep_host(inputs):
    """Fold LN1 affine + attention biases into weights on the host (fp32),
    cast weights to bf16, and return (shared input map, flags)."""
    import ml_dtypes

    BF = ml_dtypes.bfloat16
    f = {k: np.ascontiguousarray(np.asarray(v, np.float32))
         for k, v in inputs.items()}
    g1, be1 = f["g1"], f["beta1"]
    g2, be2 = f["g2"], f["beta2"]
    wq = g1[:, None] * f["W_q"]
    bq = f["b_q"] + be1 @ f["W_q"]
    wk = g1[:, None] * f["W_k"]
    wv = g1[:, None] * f["W_v"]
    bv = f["b_v"] + be1 @ f["W_v"]
    bb2 = be1 + f["b_o"] + bv @ f["W_o"]

    trivial1 = bool(np.all(g1 == 1.0))
    trivial2 = bool(np.all(g2 == 1.0) and np.all(be2 == 0.0))

    def bf(a):
        return np.ascontiguousarray(a.astype(BF))

    shared = {
        "wit": bf(f["W_it"]), "wg": bf(f["W_g"]),
        "wq": bf(wq), "wk": bf(wk), "wv": bf(wv), "wo": bf(f["W_o"]),
        "bit": bf(f["b_it"]), "bg": bf(f["b_g"]),
        "bq": bf(bq), "bb2": bf(bb2),
    }
    if not trivial1:
        shared["g1v"] = g1
    if not trivial2:
        shared["g2v"] = g2
        shared["be2v"] = be2
    return shared, trivial1, trivial2


def run(inputs, trace=False):
    shared, trivial1, trivial2 = _prep_host(inputs)
    nc = _get_nc(trivial1, trivial2)
    x = np.ascontiguousarray(np.asarray(inputs["x"], np.float32))
    adj = np.ascontiguousarray(np.asarray(inputs["adj"], np.float32))
    in_maps = []
    for b in range(NCORES):
        m = dict(shared)
        m["x"] = x[b]
        m["adj"] = adj[b]
        in_maps.append(m)
    res = run_bass_kernel_spmd(nc, in_maps, core_ids=list(range(NCORES)),
                               trace=trace)
    out = np.stack([res.results[b]["out"] for b in range(NCORES)], axis=0)
    return out, res


def kernel(**inputs):
    out, _ = run(inputs, trace=False)
    return out
